# revision 21
# baseline (speedup 1.0000x reference)
"""Trainium2 Bass kernel for 3-layer heterogeneous GraphSAGE (EntityGraphNN).

8 NeuronCores, SPMD single program:
  - Destination sharding with stripe interleave: global stripe g (2048 dst
    rows) -> core g%8, local stripe g//8. Node tables are fp16 in HBM in a
    permuted row order r(n) = shard_base + local_stripe*2048 + p*16 + t
    (p = n%128, t = (n//128)%16) so per-stripe transposed-z tiles write
    the table with fully contiguous 4KB-per-partition DMAs.
  - h[src] gathers fetch PAIRS of fp16 rows (elem 256B, the minimum legal
    dma_gather element): run slots are parity-segmented so each 128-slot
    chunk uses one fixed half of its gathered pairs as the matmul lhsT.
  - Edges exact-packed per (stripe, src-pair-bank, sub) run; structure is
    the max across cores (SPMD); per-core counts via reg_load ->
    num_idxs_reg; every slot fetches (pads fetch pair 0, masked by
    dstl=-1/ivc=0) so gather buffers are always fully overwritten.
  - Per run, one DVE op scales gathered rows by 1/deg (mean fold) and
    batched DVE ops build one-hot matrices; all operands use pair-
    duplicated values so the last AP dim is packed -> 2x 16-bit DVE mode.
  - Scatter via PE: agg_T[64, 512-quarter] += G_half^T @ onehot windows,
    fp16 (1 cycle/row). PSUM quarters zeroed by ACT memzero.
  - z computed transposed per stripe: z_T = sum_et Wl_et^T @ mean_et_T +
    Wrc^T @ hT_own (+bias); ReLU on ACT; per-tile PE transposes produce
    the row-major table shard; logits via w_cls^T @ z_T on the last layer.
  - Input projections computed sharded + AllGathered (no replicated work).
  - fp16 AllGathers (Shared outputs): proj(r,j,c), L0(c,r,j), L1(r,j);
    pass order c,r,j; no global barriers (tile data deps carry ordering).
"""
import numpy as np

HID = 64
P = 128
S = 16            # tiles per stripe
SW = P * S        # rows per stripe (2048)
BANK = 32768
N_CORES = 8
MAXC = 24         # max chunks per gather run (sub-run split)
OHB = 16          # one-hot entries per DVE op

_ETYPES = {  # et: (src_kind, dst_kind)
    "rev_part": ("r", "c"),
    "monte": ("j", "c"),
    "part": ("c", "r"),
    "rev_monte": ("c", "j"),
}
_DST_ETS = {"c": ["rev_part", "monte"], "r": ["part"], "j": ["rev_monte"]}
# slot position of each et within its dst pass (for shared SBUF tags)
_ET_SLOT = {"rev_part": 0, "part": 0, "rev_monte": 0, "monte": 1}


def _ceil(a, b):
    return (a + b - 1) // b


def _perm_rows(n, spc):
    """Node id -> permuted table row (global)."""
    g = n // SW
    core = g % N_CORES
    l = g // N_CORES
    m = n % SW
    t = m // P
    p = m % P
    return core * (spc * SW) + l * SW + p * S + t


class _EtPrep:
    pass


def _prep_et(ei, spc_src, spc_dst, n_dst_real):
    """Global edge prep for one edge type: exact packing into
    (stripe, bank, sub) runs sized by the max count across cores."""
    src = ei[0].astype(np.int64)
    dst = ei[1].astype(np.int64)
    npad_src = N_CORES * spc_src * SW
    nbanks = _ceil(npad_src // 2, BANK)   # banks of 32768 PAIR rows

    cnt = np.bincount(dst, minlength=n_dst_real).astype(np.float32)
    ivc_all = (1.0 / np.maximum(cnt, 1.0))[dst]

    g = dst // SW
    core = g % N_CORES
    sl = g // N_CORES
    m = dst % SW
    rsrc = _perm_rows(src, spc_src)
    pairr = rsrc >> 1
    par = (rsrc & 1).astype(np.int64)
    bank = pairr // BANK
    idxv = (pairr - bank * BANK).astype(np.int64)

    run = sl * nbanks + bank
    nrun_slots = spc_dst * nbanks
    order = np.lexsort((m, par, run, core))
    co, runo = core[order], run[order]
    mo, idxo, ivco = m[order], idxv[order], ivc_all[order]
    paro = par[order]

    # per (core, run, parity) counts; even segment padded to the max8
    # even-count so chunks are single-parity across all cores (SPMD)
    key2 = (co * nrun_slots + runo) * 2 + paro
    cnts2 = np.bincount(key2, minlength=N_CORES * nrun_slots * 2)
    cnts2 = cnts2.reshape(N_CORES, nrun_slots, 2)
    cnts = cnts2.sum(axis=2)
    starts = np.zeros(N_CORES * nrun_slots + 1, np.int64)
    np.cumsum(cnts.reshape(-1), out=starts[1:])

    maxc = cnts.max(axis=0)
    active = np.nonzero(maxc > 0)[0]
    ES_all = np.maximum(_ceil(cnts2[:, :, 0].max(axis=0), P) * P, 0)

    # split into sub-runs of <= MAXC*P slots
    runs = []          # (sl, bank, sub, slots, idx_col_ofs, chunk_ofs, entry_ofs, entries)
    idx_cols = 0
    chunk_ofs = 0
    entry_ofs = 0
    seg_of = {}
    for rid in active:
        ES = int(ES_all[rid])
        OS = max(_ceil(int(cnts2[:, rid, 1].max()), P) * P, 0)
        if ES + OS == 0:
            ES = P
        seg_of[rid] = ES
        mx = ES + OS
        nsub = _ceil(mx, MAXC * P)
        for k in range(nsub):
            lo = k * MAXC * P
            s = min(mx - lo, MAXC * P)
            s = max(_ceil(s, P) * P, P)
            # entries: union of (chunk, tile) across cores for this sub-run
            ct = set()
            for c in range(N_CORES):
                kk = c * nrun_slots + rid
                s0, s1 = int(starts[kk]), int(starts[kk + 1])
                ne_, no_ = int(cnts2[c, rid, 0]), int(cnts2[c, rid, 1])
                pos_all = np.concatenate([
                    np.arange(ne_), ES + np.arange(no_)])
                msel = np.concatenate([
                    mo[s0:s0 + ne_], mo[s0 + ne_:s1]])
                inrange = (pos_all >= lo) & (pos_all < lo + s)
                ch = (pos_all[inrange] - lo) // P
                tl = msel[inrange] // P
                ct.update(zip(ch.tolist(), tl.tolist()))
            ent = sorted(ct)
            runs.append((int(rid // nbanks), int(rid % nbanks), k, s,
                         idx_cols, chunk_ofs, entry_ofs, ent))
            idx_cols += s // 16
            chunk_ofs += s // P
            entry_ofs += len(ent)

    tot_chunks = max(chunk_ofs, 1)
    tot_entries = max(entry_ofs, 1)
    idx_cols = max(idx_cols, 16)
    n_runs = max(len(runs), 1)

    idx_arr = np.full((N_CORES, 16, idx_cols), -1, np.int16)
    ivc_arr = np.zeros((N_CORES, P, tot_chunks), np.float32)
    dstl_arr = np.full((N_CORES, P, tot_entries), -1.0, np.float16)
    counts_arr = np.zeros((N_CORES, n_runs), np.int32)

    for ri, (r_sl, r_bank, r_sub, s, ic0, c0, e0, ent) in enumerate(runs):
        rid = r_sl * nbanks + r_bank
        ES = seg_of[rid]
        lo = r_sub * MAXC * P
        C = s // P
        for c in range(N_CORES):
            kk = c * nrun_slots + rid
            s0, s1 = int(starts[kk]), int(starts[kk + 1])
            ne_, no_ = int(cnts2[c, rid, 0]), int(cnts2[c, rid, 1])
            # run-local slot assignment: even edges at [0, ne_), interior
            # dummy-0 pad to ES, odd edges at [ES, ES+no_), trailing -1
            run_iv = np.zeros(ES, np.int64)
            run_m = np.full(ES, -1, np.int64)
            run_ic = np.zeros(ES, np.float32)
            run_iv[:ne_] = idxo[s0:s0 + ne_]
            run_m[:ne_] = mo[s0:s0 + ne_]
            run_ic[:ne_] = ivco[s0:s0 + ne_]
            if no_ > 0:
                run_iv = np.concatenate([run_iv, idxo[s0 + ne_:s1]])
                run_m = np.concatenate([run_m, mo[s0 + ne_:s1]])
                run_ic = np.concatenate([run_ic, ivco[s0 + ne_:s1]])
                cnt_n = ES + no_
            else:
                cnt_n = ne_ if ne_ > 0 else 1
            # slice this sub-run's slot range
            iv = np.full(s, -1, np.int64)
            ivq = np.zeros(s, np.float32)
            mloc1 = np.full(s, -1, np.int64)
            seg = run_iv[lo:lo + s]
            iv[:len(seg)] = seg
            mseg = run_m[lo:lo + s]
            mloc1[:len(mseg)] = mseg
            icseg = run_ic[lo:lo + s]
            ivq[:len(icseg)] = icseg
            # fetch every slot (pads fetch pair 0): the gather then fully
            # overwrites its SBUF region, so no stale/NaN data can survive
            iv[iv < 0] = 0
            counts_arr[c, ri] = s
            idx_arr[c, :, ic0:ic0 + s // 16] = iv.reshape(s // 16, 16).T
            ivc_arr[c, :, c0:c0 + C] = ivq.reshape(C, P).T
            mloc = mloc1.reshape(C, P)
            for ei_, (ch, tl) in enumerate(ent):
                mv = mloc[ch]
                rel = mv - tl * P
                col = np.where((mv >= 0) & (rel >= 0) & (rel < P),
                               rel, -1).astype(np.float16)
                dstl_arr[c, :, e0 + ei_] = col

    pr = _EtPrep()
    pr.runs = runs
    pr.seg_of = seg_of
    pr.nbanks = nbanks
    pr.idx = np.tile(idx_arr, (1, 8, 1))
    pr.ivc = ivc_arr
    pr.dstl = dstl_arr
    pr.counts = counts_arr.reshape(N_CORES, 1, n_runs)
    pr.idx_cols = idx_cols
    pr.tot_chunks = tot_chunks
    pr.tot_entries = tot_entries
    pr.n_runs = n_runs
    pr.runs_by_stripe = {}
    for ri, r in enumerate(runs):
        pr.runs_by_stripe.setdefault(r[0], []).append(ri)
    return pr


def _build(inputs):
    import concourse.bass as bass
    import concourse.mybir as mybir
    import concourse.tile as tile
    import concourse.bacc as bacc
    import jax
    from jax.sharding import Mesh, PartitionSpec, NamedSharding
    from jax.experimental.shard_map import shard_map
    from concourse.bass2jax import (_bass_exec_p, partition_id_tensor,
                                    install_neuronx_cc_hook)
    from concourse.masks import make_identity

    f32 = mybir.dt.float32
    f16 = mybir.dt.float16
    i16 = mybir.dt.int16
    i32 = mybir.dt.int32
    AF = mybir.ActivationFunctionType
    ALU = mybir.AluOpType

    x_np = {"c": np.asarray(inputs["x_cheval"], np.float32),
            "j": np.asarray(inputs["x_jockey"], np.float32),
            "r": np.asarray(inputs["x_course"], np.float32)}
    NC = x_np["c"].shape[0]
    nreal = {k: x_np[k].shape[0] for k in x_np}
    spc = {k: _ceil(_ceil(nreal[k], SW), N_CORES) for k in x_np}
    shard = {k: spc[k] * SW for k in spc}
    npad = {k: N_CORES * shard[k] for k in spc}
    din = {k: x_np[k].shape[1] + 1 for k in x_np}

    w_in_np = {}
    for k, nm in (("c", "cheval"), ("j", "jockey"), ("r", "course")):
        w = np.asarray(inputs[f"w_in_{nm}"], np.float32)
        b = np.asarray(inputs[f"b_in_{nm}"], np.float32)
        w_in_np[k] = np.concatenate([w, b.reshape(1, HID)], 0).astype(np.float16)

    w_cls = np.asarray(inputs["w_cls"], np.float32).astype(np.float16)
    b_cls = float(np.asarray(inputs["b_cls"]).reshape(-1)[0])
    eis = {et: np.asarray(inputs["ei_" + et]) for et in _ETYPES}
    NL = np.asarray(inputs["wl_part"]).shape[0]

    WL = {et: np.asarray(inputs["wl_" + et], np.float32).astype(np.float16)
          for et in _ETYPES}
    WRc, Bc = {}, {}
    for dk, ets in _DST_ETS.items():
        WRc[dk] = sum(np.asarray(inputs["wr_" + et], np.float32) for et in ets
                      ).astype(np.float16)
        Bc[dk] = sum(np.asarray(inputs["bl_" + et], np.float32) for et in ets)
    bias_nz = {dk: bool(np.any(Bc[dk])) for dk in _DST_ETS}

    prep = {}
    for et, (sk, dk) in _ETYPES.items():
        prep[et] = _prep_et(eis[et], spc[sk], spc[dk], nreal[dk])

    # shared SBUF tag slot sizes (position 0 / 1 within a dst pass)
    slot_idx = [16, 16]
    slot_chunks = [1, 1]
    slot_entries = [1, 1]
    for et in _ETYPES:
        s = _ET_SLOT[et]
        slot_idx[s] = max(slot_idx[s], prep[et].idx_cols)
        slot_chunks[s] = max(slot_chunks[s], prep[et].tot_chunks)
        slot_entries[s] = max(slot_entries[s], prep[et].tot_entries)

    # per-core xo: [din+1, shard] fp16, columns in local (stripe-major) order
    xo_np = []
    for c in range(N_CORES):
        m = {}
        for k in x_np:
            arr = np.zeros((din[k], shard[k]), np.float16)
            arr[-1, :] = 1.0
            for l in range(spc[k]):
                g0 = (l * N_CORES + c) * SW
                g1 = min(g0 + SW, nreal[k])
                if g1 > g0:
                    arr[:-1, l * SW:l * SW + (g1 - g0)] = \
                        x_np[k][g0:g1, :].T.astype(np.float16)
            m[k] = arr
        xo_np.append(m)

    iota128_v = np.broadcast_to(np.arange(P, dtype=np.float16), (P, P)).copy()

    nc = bacc.Bacc(None, num_swdge_queues=1)

    xo_t = {k: nc.declare_dram_parameter(f"xo_{k}", [din[k], shard[k]], f16,
                                         False) for k in spc}
    win_t = {k: nc.declare_dram_parameter(f"win_{k}", [din[k], HID], f16, False)
             for k in spc}
    wl_t = {et: nc.declare_dram_parameter(f"wl_{et}", [NL, HID, HID], f16,
                                          False) for et in _ETYPES}
    wrc_t = {dk: nc.declare_dram_parameter(f"wrc_{dk}", [NL, HID, HID], f16,
                                           False) for dk in _DST_ETS}
    bc_t = {dk: nc.declare_dram_parameter(f"bc_{dk}", [NL, 1, HID], f16, False)
            for dk in _DST_ETS if bias_nz[dk]}
    wcls_t = nc.declare_dram_parameter("wcls", [HID, 1], f16, False)
    iota_t = nc.declare_dram_parameter("iota128", [P, P], f16, False)
    idx_t = {et: nc.declare_dram_parameter(f"idx_{et}", [P, prep[et].idx_cols],
                                           i16, False) for et in _ETYPES}
    ivc_t = {et: nc.declare_dram_parameter(f"ivcq_{et}",
                                           [P, prep[et].tot_chunks], f32,
                                           False) for et in _ETYPES}
    dstl_t = {et: nc.declare_dram_parameter(f"dstl_{et}",
                                            [P, prep[et].tot_entries], f16,
                                            False) for et in _ETYPES}
    cnts_t = {et: nc.declare_dram_parameter(f"cnts_{et}",
                                            [1, prep[et].n_runs], i32, False)
              for et in _ETYPES}
    out_t = nc.declare_dram_parameter("out", [shard["c"], 1], f32, True)

    vmax = {"c": NL - 1, "r": NL, "j": NL}

    with tile.TileContext(nc) as tc:
        with (
            tc.tile_pool(name="wpool", bufs=1) as wpool,
            tc.tile_pool(name="mpool", bufs=1) as mpool,
            tc.tile_pool(name="gpool", bufs=2) as gpool,
            tc.tile_pool(name="ohpool", bufs=2) as ohpool,
            tc.tile_pool(name="pool", bufs=2) as pool,
            tc.tile_pool(name="psA", bufs=1, space="PSUM") as psA,
            tc.tile_pool(name="psT", bufs=1, space="PSUM") as psT,
        ):
            ag_t, hT_t, sh_t = {}, {}, {}
            for k in spc:
                for v in range(vmax[k]):
                    ag_t[(k, v)] = nc.dram_tensor(
                        f"ag_{k}{v}", [npad[k], HID], f32, addr_space="Shared")
                    sh_t[(k, v)] = nc.dram_tensor(
                        f"sh_{k}{v}", [shard[k], HID], f32)
                for v in range(NL):
                    hT_t[(k, v)] = nc.dram_tensor(
                        f"hT_{k}{v}", [HID, shard[k]], f16)

            ident = wpool.tile([P, P], f16, tag="ident", name="ident")
            make_identity(nc, ident[:])
            iota = wpool.tile([P, P], f16, tag="iota", name="iota")
            nc.sync.dma_start(iota[:], iota_t[:])
            wcls_sb = wpool.tile([HID, 1], f16, tag="wcls", name="wcls")
            nc.sync.dma_start(wcls_sb[:], wcls_t[:])
            win_sb, wl_sb, wrc_sb, bc_sb = {}, {}, {}, {}
            for k in spc:
                win_sb[k] = wpool.tile([din[k], HID], f16, tag=f"win{k}",
                                       name=f"win{k}")
                nc.sync.dma_start(win_sb[k][:], win_t[k][:])
            for et in _ETYPES:
                for l in range(NL):
                    wl_sb[(et, l)] = wpool.tile([HID, HID], f16,
                                                tag=f"wl{et}{l}",
                                                name=f"wl{et}{l}")
                    nc.sync.dma_start(wl_sb[(et, l)][:], wl_t[et][l])
            for dk in _DST_ETS:
                for l in range(NL):
                    wrc_sb[(dk, l)] = wpool.tile([HID, HID], f16,
                                                 tag=f"wrc{dk}{l}",
                                                 name=f"wrc{dk}{l}")
                    nc.sync.dma_start(wrc_sb[(dk, l)][:], wrc_t[dk][l])
                    if bias_nz[dk]:
                        bc_sb[(dk, l)] = wpool.tile([1, HID], f16,
                                                    tag=f"bc{dk}{l}",
                                                    name=f"bc{dk}{l}")
                        nc.sync.dma_start(bc_sb[(dk, l)][:], bc_t[dk][l])
            cnts_sb = {}
            for et in _ETYPES:
                cnts_sb[et] = wpool.tile([1, prep[et].n_runs], i32,
                                         tag=f"cnts{et}", name=f"cnts{et}")
                nc.sync.dma_start(cnts_sb[et][:], cnts_t[et][:])
            ones_sb = None
            if any(bias_nz.values()):
                ones_sb = wpool.tile([1, 512], f16, tag="ones", name="ones")
                nc.gpsimd.memset(ones_sb[:], 1.0)

            # memset gather bufs once (gathered padding slots must be finite)
            for sslot in range(2):
                for _ in range(2):
                    t_ = gpool.tile([P, MAXC * HID], f32, tag=f"g{sslot}")
                    nc.gpsimd.memset(t_[:], 0.0)

            qrot = [0]
            idx_sb, ivc_sb, dstl_sb = {}, {}, {}
            cnt_regs = [nc.gpsimd.alloc_register(f"cntr{i}")
                        for i in range(12)]

            def load_et_meta(ets):
                for et in ets:
                    pr = prep[et]
                    s = _ET_SLOT[et]
                    it = mpool.tile([P, slot_idx[s]], i16, tag=f"idx{s}")
                    nc.sync.dma_start(it[:, :pr.idx_cols], idx_t[et][:])
                    idx_sb[et] = it
                    iv = mpool.tile([P, slot_chunks[s]], f32, tag=f"ivc{s}")
                    nc.sync.dma_start(iv[:, :pr.tot_chunks], ivc_t[et][:])
                    ivc_sb[et] = iv
                    dl = mpool.tile([P, slot_entries[s]], f16, tag=f"dstl{s}")
                    nc.sync.dma_start(dl[:, :pr.tot_entries], dstl_t[et][:])
                    dstl_sb[et] = dl

            def emit_tail(dk, si, zT_sb, l, write_table):
                nc.sync.dma_start(hT_t[(dk, l + 1)][:, si * SW:(si + 1) * SW],
                                  zT_sb[:])
                if not write_table:
                    return
                tp = psT.tile([P, S * HID], f16, space="PSUM", tag="tp")
                for t in range(S):
                    nc.tensor.transpose(
                        out=tp[:, t * HID:(t + 1) * HID],
                        in_=zT_sb[:, t * P:(t + 1) * P],
                        identity=ident[:HID, :HID])
                zrow = pool.tile([P, S * HID], f32, tag="zrow")
                nc.scalar.copy(zrow[:], tp[:])
                nc.sync.dma_start(
                    sh_t[(dk, l + 1)][:][si * SW:(si + 1) * SW, :]
                    .rearrange("(p t) f -> p t f", p=P),
                    zrow[:].rearrange("p (t f) -> p t f", f=HID))

            def emit_scatter(et, si, l, aggs):
                pr = prep[et]
                sk = _ETYPES[et][0]
                sslot = _ET_SLOT[et]
                tab = ag_t[(sk, l)]
                rlist = pr.runs_by_stripe.get(si, [])
                last_by_q = {}
                for ri in rlist:
                    for ei_, (ch, tl) in enumerate(pr.runs[ri][7]):
                        last_by_q[tl // 4] = (ri, ei_)
                for ri in rlist:
                    (r_sl, r_bank, r_sub, slots, ic0, c0, e0, ent) = pr.runs[ri]
                    C = slots // P
                    ne = len(ent)
                    reg = cnt_regs[qrot[0] % len(cnt_regs)]
                    nc.gpsimd.reg_load(reg, cnts_sb[et][0:1, ri:ri + 1])
                    graw = gpool.tile([P, MAXC * HID], f32, tag=f"g{sslot}")
                    b_lo = r_bank * BANK
                    b_hi = min(b_lo + BANK, npad[sk])
                    nc.gpsimd.dma_gather(
                        out_ap=graw[:, :C * HID].rearrange(
                            "p (c f) -> p c f", f=HID),
                        in_ap=tab[:][b_lo:b_hi, :],
                        idxs_ap=idx_sb[et][:, ic0:ic0 + slots // 16],
                        num_idxs=slots, num_idxs_reg=reg,
                        elem_size=HID, single_packet=False,
                        queue_num=0)
                    qrot[0] += 1
                    g16 = gpool.tile([P, MAXC * HID], f16, tag=f"g16{sslot}")
                    nc.vector.tensor_tensor(
                        out=g16[:, :C * HID].rearrange("p (c f) -> p c f",
                                                       f=HID),
                        in0=graw[:, :C * HID].rearrange("p (c f) -> p c f",
                                                        f=HID),
                        in1=ivc_sb[et][:, c0:c0 + C].unsqueeze(2)
                        .to_broadcast([P, C, HID]),
                        op=ALU.mult)
                    for eb in range(0, ne, OHB):
                        nb = min(OHB, ne - eb)
                        oh = ohpool.tile([P, OHB * P], f16, tag=f"oh{sslot}")
                        nc.vector.tensor_tensor(
                            out=oh[:, :nb * P].rearrange("p (e w) -> p e w",
                                                         w=P),
                            in0=dstl_sb[et][:, e0 + eb:e0 + eb + nb]
                            .unsqueeze(2).to_broadcast([P, nb, P]),
                            in1=iota[:].unsqueeze(1).to_broadcast([P, nb, P]),
                            op=ALU.is_equal)
                        for j in range(nb):
                            ch, tl = ent[eb + j]
                            q, tq = divmod(tl, 4)
                            stop = (last_by_q.get(q) == (ri, eb + j))
                            nc.tensor.matmul(
                                out=aggs[q][:, tq * P:(tq + 1) * P],
                                lhsT=g16[:, ch * HID:(ch + 1) * HID],
                                rhs=oh[:, j * P:(j + 1) * P],
                                start=False, stop=stop,
                                skip_group_check=True)

            def do_proj(k):
                for si in range(spc[k]):
                    xoT = pool.tile([din[k], SW], f16, tag="xoT")
                    nc.sync.dma_start(xoT[:din[k], :],
                                      xo_t[k][:, si * SW:(si + 1) * SW])
                    zT_sb = pool.tile([HID, SW], f16, tag="zT")
                    for q in range(4):
                        zq = psA.tile([HID, 512], f32, space="PSUM",
                                      tag=f"agg{q}")
                        nc.tensor.matmul(out=zq[:],
                                         lhsT=win_sb[k][:],
                                         rhs=xoT[:din[k],
                                                 q * 512:(q + 1) * 512],
                                         start=True, stop=True,
                                         skip_group_check=True)
                        nc.scalar.activation(zT_sb[:, q * 512:(q + 1) * 512],
                                             zq[:], AF.Relu)
                    emit_tail(k, si, zT_sb, -1, True)

            def do_layer(l):
                last = (l == NL - 1)
                dks = ["c"] if last else ["c", "r", "j"]
                for dk in dks:
                    ets = _DST_ETS[dk]
                    load_et_meta(ets)
                    for si in range(spc[dk]):
                        hTt = pool.tile([HID, SW], f16, tag="hTt")
                        nc.sync.dma_start(
                            hTt[:], hT_t[(dk, l)][:, si * SW:(si + 1) * SW])
                        means = {}
                        for et in ets:
                            aggs = [psA.tile([HID, 512], f32, space="PSUM",
                                             tag=f"agg{q}") for q in range(4)]
                            for q in range(4):
                                nc.scalar.memzero(aggs[q][:])
                            emit_scatter(et, si, l, aggs)
                            mt = pool.tile([HID, SW], f16,
                                           tag=f"mean{_ET_SLOT[et]}")
                            for q in range(4):
                                nc.scalar.copy(mt[:, q * 512:(q + 1) * 512],
                                               aggs[q][:])
                            means[et] = mt
                        zT_sb = pool.tile([HID, SW], f16, tag="zT")
                        for q in range(4):
                            qs = slice(q * 512, (q + 1) * 512)
                            zq = psA.tile([HID, 512], f32, space="PSUM",
                                          tag=f"agg{q}")
                            for i, et in enumerate(ets):
                                nc.tensor.matmul(out=zq[:],
                                                 lhsT=wl_sb[(et, l)][:],
                                                 rhs=means[et][:, qs],
                                                 start=(i == 0), stop=False,
                                                 skip_group_check=True)
                            nc.tensor.matmul(out=zq[:],
                                             lhsT=wrc_sb[(dk, l)][:],
                                             rhs=hTt[:, qs],
                                             start=False,
                                             stop=not bias_nz[dk],
                                             skip_group_check=True)
                            if bias_nz[dk]:
                                nc.tensor.matmul(out=zq[:],
                                                 lhsT=bc_sb[(dk, l)][:],
                                                 rhs=ones_sb[:],
                                                 start=False, stop=True,
                                                 skip_group_check=True)
                            nc.scalar.activation(zT_sb[:, qs], zq[:], AF.Relu)
                        if not last:
                            emit_tail(dk, si, zT_sb, l,
                                      not (l == NL - 2 and dk == "c"))
                        else:
                            osb = pool.tile([1, SW], f32, tag="osb")
                            for q in range(4):
                                qs = slice(q * 512, (q + 1) * 512)
                                lg = psT.tile([1, 512], f32, space="PSUM",
                                              tag="lg")
                                nc.tensor.matmul(out=lg[:],
                                                 lhsT=wcls_sb[:],
                                                 rhs=zT_sb[:, qs],
                                                 start=True, stop=True,
                                                 skip_group_check=True)
                                if b_cls != 0.0:
                                    nc.scalar.activation(osb[:, qs], lg[:],
                                                         AF.Copy, bias=b_cls)
                                else:
                                    nc.scalar.copy(osb[:, qs], lg[:])
                            nc.sync.dma_start(
                                out_t[:][si * SW:(si + 1) * SW, :]
                                .rearrange("s o -> o s"), osb[:])
                    if not last and not (l == NL - 2 and dk == "c"):
                        nc.gpsimd.collective_compute(
                            "AllGather", mybir.AluOpType.bypass,
                            ins=[sh_t[(dk, l + 1)][:]],
                            outs=[ag_t[(dk, l + 1)][:]],
                            replica_groups=[list(range(N_CORES))])

            for k in ["r", "j", "c"]:
                do_proj(k)
                nc.gpsimd.collective_compute(
                    "AllGather", mybir.AluOpType.bypass,
                    ins=[sh_t[(k, 0)][:]], outs=[ag_t[(k, 0)][:]],
                    replica_groups=[list(range(N_CORES))])

            for l in range(NL):
                do_layer(l)

    nc.finalize()

    in_maps = []
    for c in range(N_CORES):
        m = {}
        for k in spc:
            m[f"xo_{k}"] = xo_np[c][k]
            m[f"win_{k}"] = w_in_np[k]
        for et in _ETYPES:
            pr = prep[et]
            m[f"idx_{et}"] = _padcols(pr.idx[c], pr.idx_cols, -1)
            m[f"ivcq_{et}"] = _padcols(pr.ivc[c], pr.tot_chunks)
            m[f"dstl_{et}"] = _padcols(pr.dstl[c], pr.tot_entries, -1.0)
            m[f"cnts_{et}"] = pr.counts[c]
            m[f"wl_{et}"] = WL[et]
        for dk in _DST_ETS:
            m[f"wrc_{dk}"] = WRc[dk]
            if bias_nz[dk]:
                m[f"bc_{dk}"] = Bc[dk].reshape(NL, 1, HID).astype(np.float16)
        m["wcls"] = w_cls
        m["iota128"] = iota128_v
        in_maps.append(m)

    return nc, in_maps, spc, shard, NC


def kernel(**inputs):
    import concourse.mybir as mybir
    import jax
    from jax.sharding import Mesh, PartitionSpec, NamedSharding
    from jax.experimental.shard_map import shard_map
    from concourse.bass2jax import (_bass_exec_p, partition_id_tensor,
                                    install_neuronx_cc_hook)

    nc, in_maps, spc, shard, NC = _build(inputs)

    install_neuronx_cc_hook()
    partition_name = nc.partition_id_tensor.name if nc.partition_id_tensor else None
    in_names, out_names, out_avals, zero_outs = [], [], [], []
    for alloc in nc.m.functions[0].allocations:
        if not isinstance(alloc, mybir.MemoryLocationSet):
            continue
        name = alloc.memorylocations[0].name
        if alloc.kind == "ExternalInput":
            if name != partition_name:
                in_names.append(name)
        elif alloc.kind == "ExternalOutput":
            out_names.append(name)
            shape = tuple(alloc.tensor_shape)
            dtype = mybir.dt.np(alloc.dtype)
            out_avals.append(jax.core.ShapedArray(shape, dtype))
            zero_outs.append(np.zeros(shape, dtype))
    n_params = len(in_names)
    all_in = list(in_names) + list(out_names)
    if partition_name is not None:
        all_in.append(partition_name)

    def _body(*args):
        operands = list(args)
        if partition_name is not None:
            operands.append(partition_id_tensor())
        outs = _bass_exec_p.bind(
            *operands, out_avals=tuple(out_avals), in_names=tuple(all_in),
            out_names=tuple(out_names), lowering_input_output_aliases=(),
            sim_require_finite=False, sim_require_nnan=False, nc=nc)
        return tuple(outs)

    devices = jax.devices()[:N_CORES]
    mesh = Mesh(np.asarray(devices), ("core",))
    specs = (PartitionSpec("core"),)
    sharded = jax.jit(
        shard_map(_body, mesh=mesh, in_specs=specs * (n_params + len(out_names)),
                  out_specs=specs * len(out_names), check_rep=False),
        keep_unused=True)
    per_core = [[np.asarray(m[n]) for n in in_names] for m in in_maps]
    concat_in = [np.concatenate([per_core[c][i] for c in range(N_CORES)], axis=0)
                 for i in range(n_params)]
    concat_zero = [np.zeros((N_CORES * z.shape[0], *z.shape[1:]), z.dtype)
                   for z in zero_outs]
    shd = NamedSharding(mesh, PartitionSpec("core"))
    dev_in = [
        jax.make_array_from_callback(a.shape, shd, lambda idx, a=a: a[idx])
        for a in concat_in + concat_zero
    ]
    outs = sharded(*dev_in)
    jax.block_until_ready(outs)
    global _LAST
    _LAST = (sharded, dev_in)
    oi = out_names.index("out")
    res = np.asarray(outs[oi]).reshape(N_CORES, shard["c"])
    full = np.zeros((NC,), np.float32)
    n = np.arange(NC)
    g = n // SW
    full[n] = res[g % N_CORES, (g // N_CORES) * SW + (n % SW)]
    return full.reshape(NC, 1)


def _padcols(a, w, fill=0):
    if a.shape[1] == w:
        return np.ascontiguousarray(a)
    out = np.full((a.shape[0], w), fill, a.dtype)
    out[:, :a.shape[1]] = a
    return out


_LAST = None


def hw_time_ns(n=30):
    """Re-invoke the compiled sharded kernel on staged device inputs and
    return the minimum end-to-end wall time in ns."""
    import jax
    import time
    sharded, dev_in = _LAST
    jax.block_until_ready(sharded(*dev_in))
    best = float("inf")
    for _ in range(n):
        t0 = time.perf_counter_ns()
        jax.block_until_ready(sharded(*dev_in))
        best = min(best, time.perf_counter_ns() - t0)
    return best
